# revision 6
# baseline (speedup 1.0000x reference)
"""MultiHeadAttention Trainium2 kernel (B=4, S=2048, D=1024, H=16, causal).

Sharding: 8 cores = batch(4) x head-group(2). Each core computes its batch's
attention for 8 heads (e-slice of 512) plus the partial out-projection for its
d-slice; host sums the two partials per batch and adds bo.

v2: attention operands in bf16, scores matmuls for a head PAIR run
concurrently via PE row-tiling (heads 2t/2t+1 live in partitions 0:64/64:128
of e-tile t, so their K=64 contractions occupy disjoint row groups), and the
per-k-tile loop is software-pipelined (scores of k-tile kt+1 issue before the
attn@V of kt) so the PE never stalls on the Exp activation. The exp stream on
the ACT engine is the phase-2 floor (~123us); projections stay float32r.

Layouts (per core):
  xT   [D=1024, S=2048] f32r   x[b].T  (d on partitions, 8 tiles)
  QT/KT in SBUF as [e=512, s=2048] bf16 (4 partition tiles, head pair p in
       tile p: head 2p at partitions 0:64, 2p+1 at 64:128)
  V in SBUF as [s, 8 heads, 65] bf16 (64 v-cols + ones col for the denom)
  attention in "scores-transposed" orientation: ST[k, q] = (K Q^T),
  ET = exp(ST/8) masked, out_aug[hd(+den), q] = V_aug^T-contracted with ET,
  normalize by reciprocal of the denominator row, out-proj from [d, s] bf16.
"""

import numpy as np

B, S, D, H = 4, 2048, 1024, 16
HD = D // H  # 64
NCORES = 8
HPG = 8          # heads per group (per core)
EP = HPG * HD    # 512, e-slice width per core
SCH = 512        # s-chunk width (q chunks, projection n chunks)
NSC = S // SCH   # 4
NST = S // 128   # 16 s-tiles
NDT = D // 128   # 8 d-tiles
NET = EP // 128  # 4 e-tiles per core (= head pairs)

_cache = {}


def _build_program(with_pad, with_bias=False, repeat=1, phases=(1, 2, 3)):
    import concourse.tile as tile
    from concourse import bacc, mybir

    f32 = mybir.dt.float32
    bdt = mybir.dt.bfloat16
    Exp = mybir.ActivationFunctionType.Exp

    # mdt: dtype for projection matmul operands. float32r is the single-pass
    # PE mode (4x faster than fp32 at N>=256); producers must declare it.
    mdt = mybir.dt.float32r

    def mm(out_ap, lhsT, rhs, **kw):
        nc.tensor.matmul(out_ap, lhsT, rhs, **kw)

    nc = bacc.Bacc("TRN2", target_bir_lowering=False)

    xT = nc.dram_tensor("xT", (D, S), mdt, kind="ExternalInput")
    wqT = nc.dram_tensor("wqT", (D, EP), mdt, kind="ExternalInput")
    wkT = nc.dram_tensor("wkT", (D, EP), mdt, kind="ExternalInput")
    wvT = nc.dram_tensor("wvT", (D, EP), mdt, kind="ExternalInput")
    woT = nc.dram_tensor("woT", (EP, D), bdt, kind="ExternalInput")
    if with_bias:
        bq = nc.dram_tensor("bq", (EP,), f32, kind="ExternalInput")
        bk = nc.dram_tensor("bk", (EP,), f32, kind="ExternalInput")
        bv = nc.dram_tensor("bv", (EP,), f32, kind="ExternalInput")
    cmask = nc.dram_tensor("cmask", (2, 128, 2, 256), bdt, kind="ExternalInput")
    if with_pad:
        padm = nc.dram_tensor("padm", (NST, 128), f32, kind="ExternalInput")
    out = nc.dram_tensor("out", (S, D), f32, kind="ExternalOutput")

    xT_t = xT.rearrange("(dt p) s -> p dt s", p=128)
    wqT_t = wqT.rearrange("(dt p) e -> p dt e", p=128)
    wkT_t = wkT.rearrange("(dt p) e -> p dt e", p=128)
    wvT_t = wvT.rearrange("(dt p) e -> p dt e", p=128)
    woT_t = woT.rearrange("(dt p) e -> p dt e", p=128)

    with tile.TileContext(nc) as tc:
        with tc.tile_pool(name="persist", bufs=1) as pp:
            # persistent SBUF tensors (live across phases)
            QT = pp.tile([128, NET, S], bdt)    # [e-tile, s]
            KT = pp.tile([128, NET, S], bdt)
            V = pp.tile([128, NST, HPG, HD + 1], bdt)  # ones col at index HD
            cm = pp.tile([128, 2, 2, 256], bdt)
            if with_bias:
                bq_sb = pp.tile([128, NET], f32)
                bk_sb = pp.tile([128, NET], f32)
                bv_sb = pp.tile([64, HPG], f32)
            if with_pad:
                pad_sb = pp.tile([128, NST], f32)
                nc.sync.dma_start(out=pad_sb, in_=padm.rearrange("t p -> p t"))

            for pat in range(2):
                nc.sync.dma_start(out=cm[:, pat, :, :], in_=cmask[pat])
            if with_bias:
                nc.sync.dma_start(
                    out=bq_sb, in_=bq.rearrange("(t p) -> p t", p=128))
                nc.sync.dma_start(
                    out=bk_sb, in_=bk.rearrange("(t p) -> p t", p=128))
                nc.sync.dma_start(
                    out=bv_sb, in_=bv.rearrange("(h p) -> p h", p=64))
            for st in range(NST):
                nc.vector.memset(V[:, st, :, HD:HD + 1], 1.0)

            for _rep in range(repeat):
                # ---------------- Phase 1: projections ----------------
                if 1 not in phases:
                    continue
                with tc.tile_pool(name="ph1", bufs=1) as wp, \
                     tc.tile_pool(name="ph1x", bufs=2) as xp, \
                     tc.tile_pool(name="ps1", bufs=4, space="PSUM") as ps1:
                    wq_sb = wp.tile([128, NDT, EP], mdt)
                    wk_sb = wp.tile([128, NDT, EP], mdt)
                    wv_sb = wp.tile([128, NDT, EP], mdt)
                    # per-d-chunk DMAs: the first projection matmuls only
                    # wait on their own 256KB slice, not the full 2MB
                    for dt in range(NDT):
                        nc.sync.dma_start(
                            out=wq_sb[:, dt, :], in_=wqT_t[:, dt, :])
                        nc.sync.dma_start(
                            out=wk_sb[:, dt, :], in_=wkT_t[:, dt, :])
                        nc.sync.dma_start(
                            out=wv_sb[:, dt, :], in_=wvT_t[:, dt, :])

                    for sc in range(NSC):
                        xs = xp.tile([128, NDT, SCH], mdt, tag="xs")
                        for dt in range(NDT):
                            nc.sync.dma_start(
                                out=xs[:, dt, :],
                                in_=xT_t[:, dt, sc * SCH:(sc + 1) * SCH])
                        # QT / KT e-tiles: out[e128, s512] = sum_d wT[d,e] x^T[d,s]
                        for et in range(NET):
                            psq = ps1.tile([128, SCH], f32, tag="psp")
                            for dt in range(NDT):
                                mm(
                                    psq,
                                    wq_sb[:, dt, et * 128:(et + 1) * 128],
                                    xs[:, dt, :],
                                    start=(dt == 0), stop=(dt == NDT - 1))
                            if with_bias:
                                nc.vector.tensor_scalar_add(
                                    QT[:, et, sc * SCH:(sc + 1) * SCH], psq,
                                    bq_sb[:, et:et + 1])
                            else:
                                nc.scalar.copy(
                                    QT[:, et, sc * SCH:(sc + 1) * SCH], psq)
                            psk = ps1.tile([128, SCH], f32, tag="psp")
                            for dt in range(NDT):
                                mm(
                                    psk,
                                    wk_sb[:, dt, et * 128:(et + 1) * 128],
                                    xs[:, dt, :],
                                    start=(dt == 0), stop=(dt == NDT - 1))
                            if with_bias:
                                nc.vector.tensor_scalar_add(
                                    KT[:, et, sc * SCH:(sc + 1) * SCH], psk,
                                    bk_sb[:, et:et + 1])
                            else:
                                nc.scalar.copy(
                                    KT[:, et, sc * SCH:(sc + 1) * SCH], psk)
                        # V s-tiles: out[s128, e512] = sum_d x^T[d,s] wvT[d,e]
                        for st4 in range(4):
                            st = sc * 4 + st4
                            psv = ps1.tile([128, SCH], f32, tag="psp")
                            for dt in range(NDT):
                                mm(
                                    psv,
                                    xs[:, dt, st4 * 128:(st4 + 1) * 128],
                                    wv_sb[:, dt, :],
                                    start=(dt == 0), stop=(dt == NDT - 1))
                            nc.vector.tensor_copy(
                                V[:, st, :, 0:HD],
                                psv.rearrange("p (h v) -> p h v", h=HPG))

                # ---------------- Phase 2: attention ----------------
                if 2 not in phases:
                    continue
                with tc.tile_pool(name="ph2", bufs=1) as lp, \
                     tc.tile_pool(name="et", bufs=4) as etp, \
                     tc.tile_pool(name="tmp", bufs=4) as tmpp, \
                     tc.tile_pool(name="fo", bufs=4) as fop, \
                     tc.tile_pool(name="ps_s", bufs=2, space="PSUM") as pss, \
                     tc.tile_pool(name="ps_o", bufs=2, space="PSUM") as pso:
                    OT = lp.tile([128, NET, S], bdt)   # attn out, [d, s] packed
                    wo_sb = lp.tile([128, NET, D], bdt)
                    nc.sync.dma_start(out=wo_sb, in_=woT_t)

                    for p in range(NET):               # head pair (2p, 2p+1)
                        for qc in range(NSC):
                            nkt = 4 * (qc + 1)       # causal: k-tiles 0..nkt-1
                            nfull = nkt - 4          # fully-valid k-tiles
                            q0 = qc * SCH
                            ps_o = pso.tile([128, 2, SCH], f32, tag="pso")
                            ets = [None] * nkt
                            pss_t = [None] * nkt
                            # software-pipelined: scores/exp of kt issue
                            # before attn@V of kt-1 so PE stays ahead of ACT
                            for kt in range(nkt + 1):
                                if kt < nkt:
                                    # diag tiles d2/d3 only cover q 256:512
                                    qlo = 256 if kt >= nfull + 2 else 0
                                    ps_s = pss.tile(
                                        [128, 2, SCH], f32, tag="pss")
                                    pss_t[kt] = ps_s
                                    for j in range(2):
                                        mm(
                                            ps_s[:, j, qlo:],
                                            KT[64 * j:64 * j + 64, p,
                                               kt * 128:(kt + 1) * 128],
                                            QT[64 * j:64 * j + 64, p,
                                               q0 + qlo:q0 + SCH],
                                            start=True, stop=True,
                                            tile_position=(64 * j, 0))
                                    et_t = etp.tile([128, 2, SCH], bdt,
                                                    tag="et")
                                    ets[kt] = et_t
                                    nc.scalar.activation(
                                        et_t[:, :, qlo:], ps_s[:, :, qlo:],
                                        Exp, scale=0.125)
                                    if kt >= nfull:
                                        i = kt - nfull
                                        w0 = 0 if i < 2 else 256
                                        nc.vector.tensor_mul(
                                            et_t[:, :, w0:w0 + 256],
                                            et_t[:, :, w0:w0 + 256],
                                            cm[:, i % 2, :, :])
                                    if with_pad:
                                        nc.vector.tensor_scalar_mul(
                                            et_t[:, :, qlo:],
                                            et_t[:, :, qlo:],
                                            pad_sb[:, kt:kt + 1])
                                if kt >= 1:
                                    ka = kt - 1
                                    qlo = 256 if ka >= nfull + 2 else 0
                                    for j in range(2):
                                        mm(
                                            ps_o[0:HD + 1, j, qlo:],
                                            V[:, ka, 2 * p + j, :],
                                            ets[ka][:, j, qlo:],
                                            start=(ka == 0),
                                            stop=(ka == nkt - 1),
                                            skip_group_check=True)
                            # normalize: rec = 1/denominator row (at
                            # partition 64), broadcast on the Pool engine,
                            # multiply straight into the packed OT slot
                            for j in range(2):
                                rec = tmpp.tile([1, SCH], f32, tag="rec")
                                nc.vector.reciprocal(
                                    rec, ps_o[64:65, j, :])
                                bc_sb = tmpp.tile([64, SCH], f32, tag="bcs")
                                nc.gpsimd.partition_broadcast(bc_sb, rec)
                                ot_slot = OT[64 * j:64 * j + 64, p,
                                             q0:q0 + SCH]
                                nc.vector.tensor_mul(
                                    ot_slot, ps_o[0:HD, j, :], bc_sb)
                                if with_bias:
                                    nc.vector.tensor_scalar_add(
                                        ot_slot, ot_slot,
                                        bv_sb[:, 2 * p + j:2 * p + j + 1])

                    # ---------------- Phase 3: out projection ----------------
                    if 3 not in phases:
                        continue
                    for st in range(NST):
                        for ec in range(D // SCH):
                            ps_f = pso.tile([128, 2, SCH], f32, tag="pso")
                            for dt in range(NET):
                                mm(
                                    ps_f[:, 0, :],
                                    OT[:, dt, st * 128:(st + 1) * 128],
                                    wo_sb[:, dt, ec * SCH:(ec + 1) * SCH],
                                    start=(dt == 0), stop=(dt == NET - 1))
                            fo = fop.tile([128, SCH], f32, tag="fo")
                            nc.vector.tensor_copy(fo, ps_f[:, 0, :])
                            nc.sync.dma_start(
                                out=out[st * 128:(st + 1) * 128,
                                        ec * SCH:(ec + 1) * SCH],
                                in_=fo)
    nc.compile()
    return nc


def prep_in_maps(inputs):
    import ml_dtypes

    x = np.asarray(inputs["x"], dtype=np.float32)
    mask = np.asarray(inputs["attention_mask"])
    Wq = np.asarray(inputs["Wq"], dtype=np.float32)
    Wk = np.asarray(inputs["Wk"], dtype=np.float32)
    Wv = np.asarray(inputs["Wv"], dtype=np.float32)
    Wo = np.asarray(inputs["Wo"], dtype=np.float32)
    bq = np.asarray(inputs["bq"], dtype=np.float32)
    bk = np.asarray(inputs["bk"], dtype=np.float32)
    bv = np.asarray(inputs["bv"], dtype=np.float32)
    with_pad = not bool((mask != 0).all())
    with_bias = bool(bq.any() or bk.any() or bv.any())

    # causal mask patterns for the diag k-tiles, 256-wide q windows in the
    # ST orientation [k, q]: P0: qq >= kk, P1: qq >= kk + 128; duplicated
    # on axis 2 to match the [128, 2(head), 256] et windows
    kk = np.arange(128)[:, None]
    qq = np.arange(256)[None, :]
    cmask = np.stack([
        np.repeat(((qq >= kk + 128 * pat)).astype(np.float32)[:, None, :],
                  2, axis=1)
        for pat in range(2)
    ]).astype(ml_dtypes.bfloat16)

    in_maps = []
    for c in range(NCORES):
        b, g = divmod(c, 2)
        es = slice(g * EP, (g + 1) * EP)
        m = {
            "xT": np.ascontiguousarray(x[b].T),
            "wqT": np.ascontiguousarray(Wq[es, :].T),
            "wkT": np.ascontiguousarray(Wk[es, :].T),
            "wvT": np.ascontiguousarray(Wv[es, :].T),
            "woT": np.ascontiguousarray(Wo[:, es].T).astype(ml_dtypes.bfloat16),
            "cmask": cmask,
        }
        if with_bias:
            m["bq"] = np.ascontiguousarray(bq[es])
            m["bk"] = np.ascontiguousarray(bk[es])
            m["bv"] = np.ascontiguousarray(bv[es])
        if with_pad:
            m["padm"] = np.ascontiguousarray(
                mask[b].astype(np.float32).reshape(NST, 128))
        in_maps.append(m)
    return in_maps, (with_pad, with_bias)


def kernel(**inputs):
    from concourse import bass_utils

    in_maps, (with_pad, with_bias) = prep_in_maps(inputs)
    bo = np.asarray(inputs["bo"], dtype=np.float32)

    key = ("prog", with_pad, with_bias)
    if key not in _cache:
        _cache[key] = _build_program(with_pad, with_bias)
    nc = _cache[key]

    res = bass_utils.run_bass_kernel_spmd(nc, in_maps, core_ids=list(range(NCORES)))

    final = np.empty((B, S, D), dtype=np.float32)
    for b in range(B):
        final[b] = res.results[2 * b]["out"] + res.results[2 * b + 1]["out"] + bo
    return final


# revision 9
# speedup vs baseline: 1.0992x; 1.0992x over previous
"""MultiHeadAttention Trainium2 kernel (B=4, S=2048, D=1024, H=16, causal).

Sharding: 8 cores = batch(4) x head-group(2). Each core computes its batch's
attention for 8 heads (e-slice of 512) plus the partial out-projection for its
d-slice; host sums the two partials per batch and adds bo.

v2: attention operands in bf16, scores matmuls for a head PAIR run
concurrently via PE row-tiling (heads 2t/2t+1 live in partitions 0:64/64:128
of e-tile t, so their K=64 contractions occupy disjoint row groups), and the
per-k-tile loop is software-pipelined (scores of k-tile kt+1 issue before the
attn@V of kt) so the PE never stalls on the Exp activation. The exp stream on
the ACT engine is the phase-2 floor (~123us); projections stay float32r.

Layouts (per core):
  xT   [D=1024, S=2048] f32r   x[b].T  (d on partitions, 8 tiles)
  QT/KT in SBUF as [e=512, s=2048] bf16 (4 partition tiles, head pair p in
       tile p: head 2p at partitions 0:64, 2p+1 at 64:128)
  V in SBUF as [s, 8 heads, 65] bf16 (64 v-cols + ones col for the denom)
  attention in "scores-transposed" orientation: ST[k, q] = (K Q^T),
  ET = exp(ST/8) masked, out_aug[hd(+den), q] = V_aug^T-contracted with ET,
  normalize by reciprocal of the denominator row, out-proj from [d, s] bf16.
"""

import numpy as np

B, S, D, H = 4, 2048, 1024, 16
HD = D // H  # 64
NCORES = 8
HPG = 8          # heads per group (per core)
EP = HPG * HD    # 512, e-slice width per core
SCH = 512        # s-chunk width (q chunks, projection n chunks)
NSC = S // SCH   # 4
NST = S // 128   # 16 s-tiles
NDT = D // 128   # 8 d-tiles
NET = EP // 128  # 4 e-tiles per core (= head pairs)

_cache = {}


def _build_program(with_pad, with_bias=False, repeat=1, phases=(1, 2, 3)):
    import concourse.tile as tile
    from concourse import bacc, mybir

    f32 = mybir.dt.float32
    bdt = mybir.dt.bfloat16
    Exp = mybir.ActivationFunctionType.Exp

    # mdt: dtype for projection matmul operands. float32r is the single-pass
    # PE mode (4x faster than fp32 at N>=256); producers must declare it.
    mdt = mybir.dt.float32r

    def mm(out_ap, lhsT, rhs, **kw):
        nc.tensor.matmul(out_ap, lhsT, rhs, **kw)

    nc = bacc.Bacc("TRN2", target_bir_lowering=False)

    xT = nc.dram_tensor("xT", (D, S), mdt, kind="ExternalInput")
    wqT = nc.dram_tensor("wqT", (D, EP), mdt, kind="ExternalInput")
    wkT = nc.dram_tensor("wkT", (D, EP), mdt, kind="ExternalInput")
    wvT = nc.dram_tensor("wvT", (D, EP), mdt, kind="ExternalInput")
    woT = nc.dram_tensor("woT", (EP, D), bdt, kind="ExternalInput")
    if with_bias:
        bq = nc.dram_tensor("bq", (EP,), f32, kind="ExternalInput")
        bk = nc.dram_tensor("bk", (EP,), f32, kind="ExternalInput")
        bv = nc.dram_tensor("bv", (EP,), f32, kind="ExternalInput")
    cmask = nc.dram_tensor("cmask", (2, 128, 2, 256), bdt, kind="ExternalInput")
    if with_pad:
        padm = nc.dram_tensor("padm", (NST, 128), f32, kind="ExternalInput")
    out = nc.dram_tensor("out", (S, D), bdt, kind="ExternalOutput")

    xT_t = xT.rearrange("(dt p) s -> p dt s", p=128)
    wqT_t = wqT.rearrange("(dt p) e -> p dt e", p=128)
    wkT_t = wkT.rearrange("(dt p) e -> p dt e", p=128)
    wvT_t = wvT.rearrange("(dt p) e -> p dt e", p=128)
    woT_t = woT.rearrange("(dt p) e -> p dt e", p=128)

    with tile.TileContext(nc) as tc:
        with tc.tile_pool(name="persist", bufs=1) as pp:
            # persistent SBUF tensors (live across phases)
            QT = pp.tile([128, NET, S], bdt)    # [e-tile, s]
            KT = pp.tile([128, NET, S], bdt)
            V = pp.tile([128, NST, HPG, HD + 1], bdt)  # ones col at index HD
            cm = pp.tile([128, 2, 2, 256], bdt)
            if with_bias:
                bq_sb = pp.tile([128, NET], f32)
                bk_sb = pp.tile([128, NET], f32)
                bv_sb = pp.tile([64, HPG], f32)
            if with_pad:
                pad_sb = pp.tile([128, NST], f32)
                nc.sync.dma_start(out=pad_sb, in_=padm.rearrange("t p -> p t"))

            for pat in range(2):
                nc.sync.dma_start(out=cm[:, pat, :, :], in_=cmask[pat])
            if with_bias:
                nc.sync.dma_start(
                    out=bq_sb, in_=bq.rearrange("(t p) -> p t", p=128))
                nc.sync.dma_start(
                    out=bk_sb, in_=bk.rearrange("(t p) -> p t", p=128))
                nc.sync.dma_start(
                    out=bv_sb, in_=bv.rearrange("(h p) -> p h", p=64))
            for st in range(NST):
                nc.vector.memset(V[:, st, :, HD:HD + 1], 1.0)

            for _rep in range(repeat):
                # ---------------- Phase 1: projections ----------------
                if 1 not in phases:
                    continue
                with tc.tile_pool(name="ph1", bufs=1) as wp, \
                     tc.tile_pool(name="ph1x", bufs=2) as xp, \
                     tc.tile_pool(name="ps1", bufs=4, space="PSUM") as ps1:
                    wq_sb = wp.tile([128, NDT, EP], mdt)
                    wk_sb = wp.tile([128, NDT, EP], mdt)
                    wv_sb = wp.tile([128, NDT, EP], mdt)
                    # per-d-chunk DMAs: the first projection matmuls only
                    # wait on their own 256KB slice, not the full 2MB
                    for dt in range(NDT):
                        nc.sync.dma_start(
                            out=wq_sb[:, dt, :], in_=wqT_t[:, dt, :])
                        nc.sync.dma_start(
                            out=wk_sb[:, dt, :], in_=wkT_t[:, dt, :])
                        nc.sync.dma_start(
                            out=wv_sb[:, dt, :], in_=wvT_t[:, dt, :])

                    for sc in range(NSC):
                        xs = xp.tile([128, NDT, SCH], mdt, tag="xs")
                        for dt in range(NDT):
                            nc.sync.dma_start(
                                out=xs[:, dt, :],
                                in_=xT_t[:, dt, sc * SCH:(sc + 1) * SCH])
                        # QT / KT e-tiles: out[e128, s512] = sum_d wT[d,e] x^T[d,s]
                        for et in range(NET):
                            psq = ps1.tile([128, SCH], f32, tag="psp")
                            for dt in range(NDT):
                                mm(
                                    psq,
                                    wq_sb[:, dt, et * 128:(et + 1) * 128],
                                    xs[:, dt, :],
                                    start=(dt == 0), stop=(dt == NDT - 1))
                            if with_bias:
                                nc.vector.tensor_scalar_add(
                                    QT[:, et, sc * SCH:(sc + 1) * SCH], psq,
                                    bq_sb[:, et:et + 1])
                            else:
                                nc.scalar.copy(
                                    QT[:, et, sc * SCH:(sc + 1) * SCH], psq)
                            psk = ps1.tile([128, SCH], f32, tag="psp")
                            for dt in range(NDT):
                                mm(
                                    psk,
                                    wk_sb[:, dt, et * 128:(et + 1) * 128],
                                    xs[:, dt, :],
                                    start=(dt == 0), stop=(dt == NDT - 1))
                            if with_bias:
                                nc.vector.tensor_scalar_add(
                                    KT[:, et, sc * SCH:(sc + 1) * SCH], psk,
                                    bk_sb[:, et:et + 1])
                            else:
                                nc.scalar.copy(
                                    KT[:, et, sc * SCH:(sc + 1) * SCH], psk)
                        # V s-tiles: out[s128, e512] = sum_d x^T[d,s] wvT[d,e]
                        for st4 in range(4):
                            st = sc * 4 + st4
                            psv = ps1.tile([128, SCH], f32, tag="psp")
                            for dt in range(NDT):
                                mm(
                                    psv,
                                    xs[:, dt, st4 * 128:(st4 + 1) * 128],
                                    wv_sb[:, dt, :],
                                    start=(dt == 0), stop=(dt == NDT - 1))
                            nc.vector.tensor_copy(
                                V[:, st, :, 0:HD],
                                psv.rearrange("p (h v) -> p h v", h=HPG))

                # ---------------- Phase 2: attention ----------------
                if 2 not in phases:
                    continue
                with tc.tile_pool(name="ph2", bufs=1) as lp, \
                     tc.tile_pool(name="et", bufs=5) as etp, \
                     tc.tile_pool(name="tmp", bufs=4) as tmpp, \
                     tc.tile_pool(name="fo", bufs=4) as fop, \
                     tc.tile_pool(name="ps_s", bufs=3, space="PSUM") as pss, \
                     tc.tile_pool(name="ps_o", bufs=1, space="PSUM") as pso:
                    OT = lp.tile([128, NET, S], bdt)   # attn out, [d, s] packed
                    wo_sb = lp.tile([128, NET, D], bdt)
                    nc.sync.dma_start(out=wo_sb, in_=woT_t)

                    def outproj(st):
                        # one s-tile of the out projection, emitted between
                        # attention blocks so its PE/DVE/DMA work hides
                        # under the ACT exp stream
                        for ec in range(D // SCH):
                            ps_f = pss.tile([128, 2, SCH], f32, tag="pss")
                            for dt in range(NET):
                                mm(
                                    ps_f[:, 0, :],
                                    OT[:, dt, st * 128:(st + 1) * 128],
                                    wo_sb[:, dt, ec * SCH:(ec + 1) * SCH],
                                    start=(dt == 0), stop=(dt == NET - 1))
                            fo = fop.tile([128, SCH], bdt, tag="fo")
                            nc.vector.tensor_copy(fo, ps_f[:, 0, :])
                            nc.sync.dma_start(
                                out=out[st * 128:(st + 1) * 128,
                                        ec * SCH:(ec + 1) * SCH],
                                in_=fo)

                    for qc in range(NSC):
                        nkt = 4 * (qc + 1)       # causal: k-tiles 0..nkt-1
                        nfull = nkt - 4          # fully-valid k-tiles
                        q0 = qc * SCH
                        for p in range(NET):     # head pair (2p, 2p+1)
                            ps_o = pso.tile([128, 2, SCH], f32, tag="pso")
                            ets = [None] * nkt
                            # software-pipelined, skew 2: scores/exp of kt
                            # issue before attn@V of kt-2 so PE stays well
                            # ahead of ACT and sem latency is hidden
                            for kt in range(nkt + 2):
                                if kt < nkt:
                                    # diag tiles d2/d3 only cover q 256:512
                                    qlo = 256 if kt >= nfull + 2 else 0
                                    ps_s = pss.tile(
                                        [128, 2, SCH], f32, tag="pss")
                                    for j in range(2):
                                        mm(
                                            ps_s[:, j, qlo:],
                                            KT[64 * j:64 * j + 64, p,
                                               kt * 128:(kt + 1) * 128],
                                            QT[64 * j:64 * j + 64, p,
                                               q0 + qlo:q0 + SCH],
                                            start=True, stop=True,
                                            tile_position=(64 * j, 0))
                                    et_t = etp.tile([128, 2, SCH], bdt,
                                                    tag="et")
                                    ets[kt] = et_t
                                    nc.scalar.activation(
                                        et_t[:, :, qlo:], ps_s[:, :, qlo:],
                                        Exp, scale=0.125)
                                    if kt >= nfull:
                                        i = kt - nfull
                                        w0 = 0 if i < 2 else 256
                                        nc.vector.tensor_mul(
                                            et_t[:, :, w0:w0 + 256],
                                            et_t[:, :, w0:w0 + 256],
                                            cm[:, i % 2, :, :])
                                    if with_pad:
                                        nc.vector.tensor_scalar_mul(
                                            et_t[:, :, qlo:],
                                            et_t[:, :, qlo:],
                                            pad_sb[:, kt:kt + 1])
                                if kt >= 2:
                                    ka = kt - 2
                                    qlo = 256 if ka >= nfull + 2 else 0
                                    for j in range(2):
                                        mm(
                                            ps_o[0:HD + 1, j, qlo:],
                                            V[:, ka, 2 * p + j, :],
                                            ets[ka][:, j, qlo:],
                                            start=(ka == 0),
                                            stop=(ka == nkt - 1),
                                            skip_group_check=True)
                            # normalize: rec = 1/denominator row (at
                            # partition 64), broadcast on the Pool engine,
                            # multiply straight into the packed OT slot
                            for j in range(2):
                                rec = tmpp.tile([1, SCH], f32, tag="rec")
                                nc.vector.reciprocal(
                                    rec, ps_o[64:65, j, :])
                                bc_sb = tmpp.tile([64, SCH], f32, tag="bcs")
                                nc.gpsimd.partition_broadcast(bc_sb, rec)
                                ot_slot = OT[64 * j:64 * j + 64, p,
                                             q0:q0 + SCH]
                                nc.vector.tensor_mul(
                                    ot_slot, ps_o[0:HD, j, :], bc_sb)
                                if with_bias:
                                    nc.vector.tensor_scalar_add(
                                        ot_slot, ot_slot,
                                        bv_sb[:, 2 * p + j:2 * p + j + 1])
                            # out projection for the previous q-chunk, one
                            # s-tile per pair-block (spread so ACT's exp
                            # backlog covers the PE time)
                            if 3 in phases and qc > 0:
                                outproj(4 * (qc - 1) + p)
                    if 3 in phases:
                        for p in range(NET):
                            outproj(4 * 3 + p)
    nc.compile()
    return nc


def prep_in_maps(inputs):
    import ml_dtypes

    x = np.asarray(inputs["x"], dtype=np.float32)
    mask = np.asarray(inputs["attention_mask"])
    Wq = np.asarray(inputs["Wq"], dtype=np.float32)
    Wk = np.asarray(inputs["Wk"], dtype=np.float32)
    Wv = np.asarray(inputs["Wv"], dtype=np.float32)
    Wo = np.asarray(inputs["Wo"], dtype=np.float32)
    bq = np.asarray(inputs["bq"], dtype=np.float32)
    bk = np.asarray(inputs["bk"], dtype=np.float32)
    bv = np.asarray(inputs["bv"], dtype=np.float32)
    with_pad = not bool((mask != 0).all())
    with_bias = bool(bq.any() or bk.any() or bv.any())

    # causal mask patterns for the diag k-tiles, 256-wide q windows in the
    # ST orientation [k, q]: P0: qq >= kk, P1: qq >= kk + 128; duplicated
    # on axis 2 to match the [128, 2(head), 256] et windows
    kk = np.arange(128)[:, None]
    qq = np.arange(256)[None, :]
    cmask = np.stack([
        np.repeat(((qq >= kk + 128 * pat)).astype(np.float32)[:, None, :],
                  2, axis=1)
        for pat in range(2)
    ]).astype(ml_dtypes.bfloat16)

    in_maps = []
    for c in range(NCORES):
        b, g = divmod(c, 2)
        es = slice(g * EP, (g + 1) * EP)
        m = {
            "xT": np.ascontiguousarray(x[b].T),
            "wqT": np.ascontiguousarray(Wq[es, :].T),
            "wkT": np.ascontiguousarray(Wk[es, :].T),
            "wvT": np.ascontiguousarray(Wv[es, :].T),
            "woT": np.ascontiguousarray(Wo[:, es].T).astype(ml_dtypes.bfloat16),
            "cmask": cmask,
        }
        if with_bias:
            m["bq"] = np.ascontiguousarray(bq[es])
            m["bk"] = np.ascontiguousarray(bk[es])
            m["bv"] = np.ascontiguousarray(bv[es])
        if with_pad:
            m["padm"] = np.ascontiguousarray(
                mask[b].astype(np.float32).reshape(NST, 128))
        in_maps.append(m)
    return in_maps, (with_pad, with_bias)


def kernel(**inputs):
    from concourse import bass_utils

    in_maps, (with_pad, with_bias) = prep_in_maps(inputs)
    bo = np.asarray(inputs["bo"], dtype=np.float32)

    key = ("prog", with_pad, with_bias)
    if key not in _cache:
        _cache[key] = _build_program(with_pad, with_bias)
    nc = _cache[key]

    res = bass_utils.run_bass_kernel_spmd(nc, in_maps, core_ids=list(range(NCORES)))

    final = np.empty((B, S, D), dtype=np.float32)
    for b in range(B):
        final[b] = (res.results[2 * b]["out"].astype(np.float32)
                    + res.results[2 * b + 1]["out"].astype(np.float32) + bo)
    return final


# revision 12
# speedup vs baseline: 1.3597x; 1.2370x over previous
"""MultiHeadAttention Trainium2 kernel (B=4, S=2048, D=1024, H=16, causal).

Sharding: 8 cores = batch(4) x head-group(2). Each core computes its batch's
attention for 8 heads (e-slice of 512) plus the partial out-projection for its
d-slice; host sums the two partials per batch and adds bo.

v2: attention operands in bf16, scores matmuls for a head PAIR run
concurrently via PE row-tiling (heads 2t/2t+1 live in partitions 0:64/64:128
of e-tile t, so their K=64 contractions occupy disjoint row groups), and the
per-k-tile loop is software-pipelined (scores of k-tile kt+1 issue before the
attn@V of kt) so the PE never stalls on the Exp activation. The exp stream on
the ACT engine is the phase-2 floor (~123us); projections stay float32r.

Layouts (per core):
  xT   [D=1024, S=2048] f32r   x[b].T  (d on partitions, 8 tiles)
  QT/KT in SBUF as [e=512, s=2048] bf16 (4 partition tiles, head pair p in
       tile p: head 2p at partitions 0:64, 2p+1 at 64:128)
  V in SBUF as [s, 8 heads, 65] bf16 (64 v-cols + ones col for the denom)
  attention in "scores-transposed" orientation: ST[k, q] = (K Q^T),
  ET = exp(ST/8) masked, out_aug[hd(+den), q] = V_aug^T-contracted with ET,
  normalize by reciprocal of the denominator row, out-proj from [d, s] bf16.
"""

import numpy as np

B, S, D, H = 4, 2048, 1024, 16
HD = D // H  # 64
NCORES = 8
HPG = 8          # heads per group (per core)
EP = HPG * HD    # 512, e-slice width per core
SCH = 512        # s-chunk width (q chunks, projection n chunks)
NSC = S // SCH   # 4
NST = S // 128   # 16 s-tiles
NDT = D // 128   # 8 d-tiles
NET = EP // 128  # 4 e-tiles per core (= head pairs)

_cache = {}


def _build_program(with_pad, with_bias=False, repeat=1, phases=(1, 2, 3)):
    import concourse.tile as tile
    from concourse import bacc, mybir

    f32 = mybir.dt.float32
    bdt = mybir.dt.bfloat16
    Exp = mybir.ActivationFunctionType.Exp

    # mdt: dtype for projection matmul operands. float32r is the single-pass
    # PE mode (4x faster than fp32 at N>=256); producers must declare it.
    mdt = mybir.dt.float32r

    def mm(out_ap, lhsT, rhs, **kw):
        nc.tensor.matmul(out_ap, lhsT, rhs, **kw)

    nc = bacc.Bacc("TRN2", target_bir_lowering=False)

    xT = nc.dram_tensor("xT", (D, S), mdt, kind="ExternalInput")
    wqT = nc.dram_tensor("wqT", (D, EP), mdt, kind="ExternalInput")
    wkT = nc.dram_tensor("wkT", (D, EP), mdt, kind="ExternalInput")
    wvT = nc.dram_tensor("wvT", (D, EP), mdt, kind="ExternalInput")
    woT = nc.dram_tensor("woT", (EP, D), bdt, kind="ExternalInput")
    if with_bias:
        bq = nc.dram_tensor("bq", (EP,), f32, kind="ExternalInput")
        bk = nc.dram_tensor("bk", (EP,), f32, kind="ExternalInput")
        bv = nc.dram_tensor("bv", (EP,), f32, kind="ExternalInput")
    cmask = nc.dram_tensor("cmask", (2, 128, 2, 256), bdt, kind="ExternalInput")
    if with_pad:
        padm = nc.dram_tensor("padm", (NST, 128), f32, kind="ExternalInput")
    out = nc.dram_tensor("out", (S, D), bdt, kind="ExternalOutput")

    xT_t = xT.rearrange("(dt p) s -> p dt s", p=128)
    wqT_t = wqT.rearrange("(dt p) e -> p dt e", p=128)
    wkT_t = wkT.rearrange("(dt p) e -> p dt e", p=128)
    wvT_t = wvT.rearrange("(dt p) e -> p dt e", p=128)
    woT_t = woT.rearrange("(dt p) e -> p dt e", p=128)

    with tile.TileContext(nc) as tc:
        with tc.tile_pool(name="persist", bufs=1) as pp:
            # persistent SBUF tensors (live across phases)
            QT = pp.tile([128, NET, S], bdt)    # [e-tile, s]
            KT = pp.tile([128, NET, S], bdt)
            V = pp.tile([128, NST, HPG, HD + 1], bdt)  # ones col at index HD
            cm = pp.tile([128, 2, 2, 256], bdt)
            if with_bias:
                bq_sb = pp.tile([128, NET], f32)
                bk_sb = pp.tile([128, NET], f32)
                bv_sb = pp.tile([64, HPG], f32)
            if with_pad:
                pad_sb = pp.tile([128, NST], f32)
                nc.sync.dma_start(out=pad_sb, in_=padm.rearrange("t p -> p t"))

            for pat in range(2):
                nc.sync.dma_start(out=cm[:, pat, :, :], in_=cmask[pat])
            if with_bias:
                nc.sync.dma_start(
                    out=bq_sb, in_=bq.rearrange("(t p) -> p t", p=128))
                nc.sync.dma_start(
                    out=bk_sb, in_=bk.rearrange("(t p) -> p t", p=128))
                nc.sync.dma_start(
                    out=bv_sb, in_=bv.rearrange("(h p) -> p h", p=64))
            for st in range(NST):
                nc.vector.memset(V[:, st, :, HD:HD + 1], 1.0)

            for _rep in range(repeat):
                # ---------------- Phase 1: projections ----------------
                if 1 not in phases:
                    continue
                with tc.tile_pool(name="ph1", bufs=1) as wp, \
                     tc.tile_pool(name="ph1x", bufs=2) as xp, \
                     tc.tile_pool(name="ps1", bufs=4, space="PSUM") as ps1:
                    wq_sb = wp.tile([128, NDT, EP], mdt)
                    wk_sb = wp.tile([128, NDT, EP], mdt)
                    wv_sb = wp.tile([128, NDT, EP], mdt)
                    # per-d-chunk DMAs: the first projection matmuls only
                    # wait on their own 256KB slice, not the full 2MB
                    for dt in range(NDT):
                        nc.sync.dma_start(
                            out=wq_sb[:, dt, :], in_=wqT_t[:, dt, :])
                        nc.sync.dma_start(
                            out=wk_sb[:, dt, :], in_=wkT_t[:, dt, :])
                        nc.sync.dma_start(
                            out=wv_sb[:, dt, :], in_=wvT_t[:, dt, :])

                    for sc in range(NSC):
                        xs = xp.tile([128, NDT, SCH], mdt, tag="xs")
                        for dt in range(NDT):
                            nc.sync.dma_start(
                                out=xs[:, dt, :],
                                in_=xT_t[:, dt, sc * SCH:(sc + 1) * SCH])
                        # QT / KT e-tiles: out[e128, s512] = sum_d wT[d,e] x^T[d,s]
                        for et in range(NET):
                            psq = ps1.tile([128, SCH], f32, tag="psp")
                            for dt in range(NDT):
                                mm(
                                    psq,
                                    wq_sb[:, dt, et * 128:(et + 1) * 128],
                                    xs[:, dt, :],
                                    start=(dt == 0), stop=(dt == NDT - 1))
                            if with_bias:
                                nc.vector.tensor_scalar_add(
                                    QT[:, et, sc * SCH:(sc + 1) * SCH], psq,
                                    bq_sb[:, et:et + 1])
                            else:
                                nc.scalar.copy(
                                    QT[:, et, sc * SCH:(sc + 1) * SCH], psq)
                            psk = ps1.tile([128, SCH], f32, tag="psp")
                            for dt in range(NDT):
                                mm(
                                    psk,
                                    wk_sb[:, dt, et * 128:(et + 1) * 128],
                                    xs[:, dt, :],
                                    start=(dt == 0), stop=(dt == NDT - 1))
                            if with_bias:
                                nc.vector.tensor_scalar_add(
                                    KT[:, et, sc * SCH:(sc + 1) * SCH], psk,
                                    bk_sb[:, et:et + 1])
                            else:
                                nc.scalar.copy(
                                    KT[:, et, sc * SCH:(sc + 1) * SCH], psk)
                        # V s-tiles: out[s128, e512] = sum_d x^T[d,s] wvT[d,e]
                        for st4 in range(4):
                            st = sc * 4 + st4
                            psv = ps1.tile([128, SCH], f32, tag="psp")
                            for dt in range(NDT):
                                mm(
                                    psv,
                                    xs[:, dt, st4 * 128:(st4 + 1) * 128],
                                    wv_sb[:, dt, :],
                                    start=(dt == 0), stop=(dt == NDT - 1))
                            nc.vector.tensor_copy(
                                V[:, st, :, 0:HD],
                                psv.rearrange("p (h v) -> p h v", h=HPG))

                # ---------------- Phase 2: attention ----------------
                if 2 not in phases:
                    continue
                with tc.tile_pool(name="ph2", bufs=1) as lp, \
                     tc.tile_pool(name="et", bufs=5) as etp, \
                     tc.tile_pool(name="tmp", bufs=4) as tmpp, \
                     tc.tile_pool(name="fo", bufs=4) as fop, \
                     tc.tile_pool(name="ps_s", bufs=3, space="PSUM") as pss, \
                     tc.tile_pool(name="ps_o", bufs=1, space="PSUM") as pso:
                    OT = lp.tile([128, NET, S], bdt)   # attn out, [d, s] packed
                    wo_sb = lp.tile([128, NET, D], bdt)
                    nc.sync.dma_start(out=wo_sb, in_=woT_t)

                    def outproj(st):
                        # one s-tile of the out projection, emitted between
                        # attention blocks so its PE work hides under the
                        # ACT exp backlog; PSUM->SBUF copy on the Pool
                        # engine so the pss rotation isn't starved by DVE
                        for ec in range(D // SCH):
                            ps_f = pss.tile([128, 2, SCH], f32, tag="pss")
                            for dt in range(NET):
                                mm(
                                    ps_f[:, 0, :],
                                    OT[:, dt, st * 128:(st + 1) * 128],
                                    wo_sb[:, dt, ec * SCH:(ec + 1) * SCH],
                                    start=(dt == 0), stop=(dt == NET - 1))
                            fo = fop.tile([128, SCH], bdt, tag="fo")
                            nc.vector.tensor_copy(fo, ps_f[:, 0, :])
                            nc.sync.dma_start(
                                out=out[st * 128:(st + 1) * 128,
                                        ec * SCH:(ec + 1) * SCH],
                                in_=fo)

                    for qc in range(NSC):
                        nkt = 4 * (qc + 1)       # causal: k-tiles 0..nkt-1
                        nfull = nkt - 4          # fully-valid k-tiles
                        q0 = qc * SCH
                        for p in range(NET):     # head pair (2p, 2p+1)
                            ps_o = pso.tile([128, 2, SCH], f32, tag="pso")
                            ets = [None] * nkt
                            # software-pipelined, skew 2: scores/exp of kt
                            # issue before attn@V of kt-2 so PE stays well
                            # ahead of ACT and sem latency is hidden
                            for kt in range(nkt + 2):
                                if kt < nkt:
                                    # diag tiles d2/d3 only cover q 256:512
                                    qlo = 256 if kt >= nfull + 2 else 0
                                    ps_s = pss.tile(
                                        [128, 2, SCH], f32, tag="pss")
                                    for j in range(2):
                                        mm(
                                            ps_s[:, j, qlo:],
                                            KT[64 * j:64 * j + 64, p,
                                               kt * 128:(kt + 1) * 128],
                                            QT[64 * j:64 * j + 64, p,
                                               q0 + qlo:q0 + SCH],
                                            start=True, stop=True,
                                            tile_position=(64 * j, 0))
                                    et_t = etp.tile([128, 2, SCH], bdt,
                                                    tag="et")
                                    ets[kt] = et_t
                                    nc.scalar.activation(
                                        et_t[:, :, qlo:], ps_s[:, :, qlo:],
                                        Exp, scale=0.125)
                                    if kt >= nfull:
                                        i = kt - nfull
                                        w0 = 0 if i < 2 else 256
                                        nc.vector.tensor_mul(
                                            et_t[:, :, w0:w0 + 256],
                                            et_t[:, :, w0:w0 + 256],
                                            cm[:, i % 2, :, :])
                                    if with_pad:
                                        nc.vector.tensor_scalar_mul(
                                            et_t[:, :, qlo:],
                                            et_t[:, :, qlo:],
                                            pad_sb[:, kt:kt + 1])
                                if kt >= 2:
                                    ka = kt - 2
                                    qlo = 256 if ka >= nfull + 2 else 0
                                    for j in range(2):
                                        mm(
                                            ps_o[0:HD + 1, j, qlo:],
                                            V[:, ka, 2 * p + j, :],
                                            ets[ka][:, j, qlo:],
                                            start=(ka == 0),
                                            stop=(ka == nkt - 1),
                                            skip_group_check=True)
                            # out projection for the previous q-chunk, one
                            # s-tile per pair-block (spread so ACT's exp
                            # backlog covers the PE time), emitted before
                            # the normalize cluster queues work on DVE
                            if 3 in phases and qc > 0:
                                outproj(4 * (qc - 1) + p)
                            # normalize: free the single ps_o bank fast via
                            # one DVE copy to SBUF, then rec = 1/denom row
                            # (partition 64), broadcast on the Pool engine,
                            # multiply into the packed OT slot
                            osb = tmpp.tile([HD + 1, 2, SCH], f32, tag="osb")
                            nc.vector.tensor_copy(osb, ps_o[0:HD + 1, :, :])
                            recs = []
                            for j in range(2):
                                rec = tmpp.tile([1, SCH], f32, tag="rec")
                                nc.vector.reciprocal(rec, osb[64:65, j, :])
                                recs.append(rec)
                            bcs = []
                            for j in range(2):
                                bc_sb = tmpp.tile([64, SCH], f32, tag="bcs")
                                nc.gpsimd.partition_broadcast(bc_sb, recs[j])
                                bcs.append(bc_sb)
                            for j in range(2):
                                ot_slot = OT[64 * j:64 * j + 64, p,
                                             q0:q0 + SCH]
                                nc.vector.tensor_mul(
                                    ot_slot, osb[0:HD, j, :], bcs[j])
                                if with_bias:
                                    nc.vector.tensor_scalar_add(
                                        ot_slot, ot_slot,
                                        bv_sb[:, 2 * p + j:2 * p + j + 1])
                    if 3 in phases:
                        for p in range(NET):
                            outproj(4 * 3 + p)
    nc.compile()
    return nc


def prep_in_maps(inputs):
    import ml_dtypes

    x = np.asarray(inputs["x"], dtype=np.float32)
    mask = np.asarray(inputs["attention_mask"])
    Wq = np.asarray(inputs["Wq"], dtype=np.float32)
    Wk = np.asarray(inputs["Wk"], dtype=np.float32)
    Wv = np.asarray(inputs["Wv"], dtype=np.float32)
    Wo = np.asarray(inputs["Wo"], dtype=np.float32)
    bq = np.asarray(inputs["bq"], dtype=np.float32)
    bk = np.asarray(inputs["bk"], dtype=np.float32)
    bv = np.asarray(inputs["bv"], dtype=np.float32)
    with_pad = not bool((mask != 0).all())
    with_bias = bool(bq.any() or bk.any() or bv.any())

    # causal mask patterns for the diag k-tiles, 256-wide q windows in the
    # ST orientation [k, q]: P0: qq >= kk, P1: qq >= kk + 128; duplicated
    # on axis 2 to match the [128, 2(head), 256] et windows
    kk = np.arange(128)[:, None]
    qq = np.arange(256)[None, :]
    cmask = np.stack([
        np.repeat(((qq >= kk + 128 * pat)).astype(np.float32)[:, None, :],
                  2, axis=1)
        for pat in range(2)
    ]).astype(ml_dtypes.bfloat16)

    in_maps = []
    for c in range(NCORES):
        b, g = divmod(c, 2)
        es = slice(g * EP, (g + 1) * EP)
        m = {
            "xT": np.ascontiguousarray(x[b].T),
            "wqT": np.ascontiguousarray(Wq[es, :].T),
            "wkT": np.ascontiguousarray(Wk[es, :].T),
            "wvT": np.ascontiguousarray(Wv[es, :].T),
            "woT": np.ascontiguousarray(Wo[:, es].T).astype(ml_dtypes.bfloat16),
            "cmask": cmask,
        }
        if with_bias:
            m["bq"] = np.ascontiguousarray(bq[es])
            m["bk"] = np.ascontiguousarray(bk[es])
            m["bv"] = np.ascontiguousarray(bv[es])
        if with_pad:
            m["padm"] = np.ascontiguousarray(
                mask[b].astype(np.float32).reshape(NST, 128))
        in_maps.append(m)
    return in_maps, (with_pad, with_bias)


def kernel(**inputs):
    from concourse import bass_utils

    in_maps, (with_pad, with_bias) = prep_in_maps(inputs)
    bo = np.asarray(inputs["bo"], dtype=np.float32)

    key = ("prog", with_pad, with_bias)
    if key not in _cache:
        _cache[key] = _build_program(with_pad, with_bias)
    nc = _cache[key]

    res = bass_utils.run_bass_kernel_spmd(nc, in_maps, core_ids=list(range(NCORES)))

    final = np.empty((B, S, D), dtype=np.float32)
    for b in range(B):
        final[b] = (res.results[2 * b]["out"].astype(np.float32)
                    + res.results[2 * b + 1]["out"].astype(np.float32) + bo)
    return final
